# revision 76
# baseline (speedup 1.0000x reference)
"""Multi-head cross-attention Trainium2 kernel (8-core SPMD, batch-parallel).

Math (matches the reference):
    q = query @ Wq + bq            [B, NQ, H*D]
    k = key   @ Wk + bk            [B, NK, H*D]
    v = key   @ Wv + bv            [B, NK, H*D]
    S[b,h,q,n] = <q_h[q]/sqrt(D), k_h[n]>  - 1e5*(1-c_mask[b,n])
    out = softmax_n(S) @ v, heads concatenated -> [B, NQ, H*D]

Strategy:
  * Data-parallel over batch: 2 batches per core.  Batches are assigned to
    the two per-core slots by ascending valid-key count so each slot gets
    its own compiled chunk count (keys are compacted host-side to "valid
    first" order and truncated to a per-slot 128-multiple capacity; the
    pad tail of the capacity is ZEROED host-side).
  * Masking is done entirely through compaction: pad keys are zero vectors
    (scores ~0, exp ~1) but contribute nothing because (a) their V rows
    are zeroed via a masked bias matrix and (b) the softmax-denominator
    column appended to V holds the validity indicator (1 valid / 0 pad),
    so the denominator is exact.  No mask bias is needed in the Exp
    instruction, which lets one ACT instruction cover TWO key chunks
    (the scalar engine's exp stream is the critical path).
  * Scores are computed transposed (S^T[n, q]) so the PV matmul needs no
    on-chip transposition of the attention matrix.  Head PAIRS are
    projected together (M=128 matmuls) and their D=64-contraction score
    matmuls run concurrently in disjoint PE row-groups.
  * Projection and score matmuls write fp16 straight to PSUM (they are
    single-shot, no accumulation) so the PSUM->SBUF casts run in the
    DVE 2x perf mode and score groups of 2 chunks fit in 2 PSUM banks.
  * Matmul pipeline in fp16 (fp32 PSUM accumulation only where matmuls
    accumulate, i.e. the PV chunks); output is written fp16 and upcast
    on the host.
"""

import math
import os

import ml_dtypes
import numpy as np

import concourse.bass as bass
import concourse.tile as tile
from concourse import bacc, mybir
from concourse.bass_utils import run_bass_kernel_spmd

# Problem constants (hardcoded per the harness contract).
B, NQ, NK = 16, 512, 1024
CQ, CV = 128, 128
H, D = 8, 64
HD = H * D
SCALE = float(np.sqrt(D))

N_CORES = 8
B_LOC = B // N_CORES  # batches per core

F32 = mybir.dt.float32
FP16 = mybir.dt.float16
NP_FP16 = np.float16

# Set by kernel() after a traced run (test harness convenience).
LAST_EXEC_TIME_NS = None

_PROGRAM_CACHE = {}


def PACK2_ORDER(chsum):
    """pack2 column layout, ordered by when the kernel needs each piece
    (the single gpsimd DMA delivers columns roughly left to right)."""
    order = [("colmask", chsum), ("wq1", 128), ("wk1", 128), ("wv", HD),
             ("bvfull", HD), ("wq2", 128), ("wk2", 128)]
    for s in range(B_LOC):
        order.append((f"bvt{s}", HD))
    order += [("wq3", 128), ("wk3", 128), ("ident", 128)]
    return order


def _build_program(chunk_cfg):
    """Build + compile the single-core Bass program (SPMD across 8 cores).

    chunk_cfg: tuple of per-slot chunk counts, len == B_LOC.
    """
    CH = list(chunk_cfg)
    CAPS = [c * 128 for c in CH]
    KCUM = [sum(CAPS[:b]) for b in range(B_LOC + 1)]  # keyT col offsets
    CCUM = [sum(CH[:b]) for b in range(B_LOC + 1)]  # chunk offsets
    capsum = KCUM[-1]
    chsum = CCUM[-1]

    nc = bacc.Bacc(
        "TRN2",
        target_bir_lowering=False,
        debug=False,
        enable_asserts=False,
        num_devices=N_CORES,
    )

    # Constant inputs ride in two packed tensors (fewer DMAs, fewer tiles,
    # fewer semaphores to tear down at program end):
    #   pack1 = [wq_p0 | wk_p0 | bq | bk]     (only what pair 0 needs: small,
    #                                          so the first Exp starts early)
    #   pack2 = [wq_p123 | wk_p123 | wv | bvmat (1+B_LOC blocks) | ident
    #            | colmask]
    # bvmat block 0 = plain bv broadcast; block 1+s = slot s tail-chunk
    # variant with the pad partitions zeroed.
    P1 = 2 * 128 + 8
    P2 = sum(w for _, w in PACK2_ORDER(chsum))
    qT_d = nc.dram_tensor("queryT", [CQ, B_LOC * NQ], FP16, kind="ExternalInput").ap()
    kT_d = nc.dram_tensor("keyT", [CV, capsum], FP16, kind="ExternalInput").ap()
    p1_d = nc.dram_tensor("pack1", [128, P1], FP16, kind="ExternalInput").ap()
    p2_d = nc.dram_tensor("pack2", [128, P2], FP16, kind="ExternalInput").ap()
    # Output layout [b, pair, sbuf_partition, qtile*128+d]: each pair's store
    # is then 128 contiguous 1KB descriptors (the host reassembles).
    out_d = nc.dram_tensor(
        "out", [B_LOC, 4, 128, 4 * 128], FP16, kind="ExternalOutput"
    ).ap()

    with tile.TileContext(nc) as tc:
        with (
            tc.tile_pool(name="const", bufs=1) as const,
            tc.tile_pool(name="expsp", bufs=3) as expsp,
            tc.tile_pool(name="ctp", bufs=2) as ctp,
            tc.tile_pool(name="cp", bufs=2) as cp,
            tc.tile_pool(name="recp", bufs=2) as recp,
            tc.tile_pool(name="ps_proj", bufs=2, space="PSUM") as ps_proj,
            tc.tile_pool(name="ps_s", bufs=2, space="PSUM") as ps_s,
            tc.tile_pool(name="ps_pv", bufs=2, space="PSUM") as ps_pv,
        ):
            # ---- ACT warmup first: trigger the exp table load while idle ----
            ones_col = const.tile([128, 1], F32, tag="ones_col")
            nc.vector.memset(ones_col[:], 1.0)
            warm_sb = const.tile([128, 8], F32, tag="warm_sb")
            nc.scalar.activation(
                warm_sb[:],
                ones_col[:].broadcast_to([128, 8]),
                mybir.ActivationFunctionType.Exp,
            )

            # ---- PE warmup on a memset tile (no DMA dependency) ----
            # Enough back-to-back matmuls to climb to the top PE p-state AND
            # keep it there through the input-DMA window (an idle PE clocks
            # back down, making the first real score matmuls 2-3x slower).
            warm_w = const.tile([128, NQ], FP16, tag="warm_w")
            nc.vector.memset(warm_w[:], 0.25)
            warm_ps = ps_proj.tile([128, NQ], F32, tag="ps")
            for w in range(8):
                nc.tensor.matmul(
                    warm_ps[:],
                    warm_w[:, 0:128],
                    warm_w[:],
                    start=True,
                    stop=True,
                )
            for w in range(8):
                nc.tensor.matmul(
                    warm_ps[:, 0:128],
                    warm_w[:, 0:128],
                    warm_w[:, 0:128],
                    start=True,
                    stop=True,
                )
            nc.vector.tensor_copy(warm_sb[:], warm_ps[:, 0:8])

            # ---- inputs / weights ----
            # Wave 1: exactly what the first pair's projections and scores
            # need, split across THREE queues (sync, vector, gpsimd) so the
            # transfers run in parallel right after the program preamble.
            # Wave 2 (slot>=1 slices, pack2) follows on the gpsimd queue.
            # Batch processing order (descending chunk count; the final pair
            # is cheap so the serial tail of the program is short).
            border = sorted(range(B_LOC), key=lambda b: -CH[b])
            b0 = border[0]
            queryT_sb = const.tile([128, B_LOC * NQ], FP16, tag="queryT_sb")
            nc.sync.dma_start(
                queryT_sb[:, b0 * NQ : (b0 + 1) * NQ], qT_d[:, b0 * NQ : (b0 + 1) * NQ]
            )
            keyT_sb = const.tile([128, capsum], FP16, tag="keyT_sb")
            nc.scalar.dma_start(
                keyT_sb[:, KCUM[b0] : KCUM[b0 + 1]], kT_d[:, KCUM[b0] : KCUM[b0 + 1]]
            )
            pack1 = const.tile([128, P1], FP16, tag="pack1")
            nc.gpsimd.dma_start(pack1[:], p1_d[:])
            pack2 = const.tile([128, P2], FP16, tag="pack2")
            nc.gpsimd.dma_start(pack2[:], p2_d[:])
            for s in border[1:]:
                nc.gpsimd.dma_start(
                    queryT_sb[:, s * NQ : (s + 1) * NQ], qT_d[:, s * NQ : (s + 1) * NQ]
                )
                nc.gpsimd.dma_start(
                    keyT_sb[:, KCUM[s] : KCUM[s + 1]], kT_d[:, KCUM[s] : KCUM[s + 1]]
                )
            # tensor_scalar requires fp32 scalars; upcast the biases once.
            bqbk = const.tile([128, 8], F32, tag="bqbk")
            nc.vector.tensor_copy(bqbk[:], pack1[:, 256:264])
            bq_sb = bqbk[:, 0:4]
            bk_sb = bqbk[:, 4:8]

            # pack2 slice table (PACK2_ORDER defines the column layout).
            p2_off = {}
            off = 0
            for nm, w in PACK2_ORDER(chsum):
                p2_off[nm] = (off, off + w)
                off += w

            def p2s(nm):
                a, z = p2_off[nm]
                return pack2[:, a:z]

            def wq_slice(p):
                return pack1[:, 0:128] if p == 0 else p2s(f"wq{p}")

            def wk_slice(p):
                return pack1[:, 128:256] if p == 0 else p2s(f"wk{p}")

            colmask_sb = p2s("colmask")
            wv_sb = p2s("wv")
            ident_sb = p2s("ident")

            # ---- projections ----
            # qT_all / kT_all hold head PAIRS: partitions 0-63 = head 2p,
            # partitions 64-127 = head 2p+1 (that is just Wx columns p*128..).
            qT_all = const.tile([128, B_LOC * 4 * NQ], FP16, tag="qT_all")
            kT_all = const.tile([128, 4 * capsum], FP16, tag="kT_all")
            # v_all: per (b, chunk): 8 heads x (64 values + validity column).
            v_all = const.tile([128, chsum * 520], FP16, tag="v_all")
            v_view = v_all[:].rearrange("p (c h x) -> p c h x", h=H, x=65)

            def emit_vcol(b):
                # Validity column: 1.0 for valid keys, 0.0 for the zero pads.
                # Emitted per batch (not up front) so the DVE queue is not
                # blocked early behind the colmask DMA.
                nc.vector.tensor_copy(
                    v_view[:, CCUM[b] : CCUM[b + 1], :, 64],
                    colmask_sb[:, CCUM[b] : CCUM[b + 1]]
                    .unsqueeze(2)
                    .broadcast_to([128, CH[b], H]),
                )

            def emit_qk_proj(b, p):
                cap = CAPS[b]
                ps = ps_proj.tile([128, NQ], F32, tag="ps")
                nc.tensor.matmul(
                    ps[:],
                    wq_slice(p),
                    queryT_sb[:, b * NQ : (b + 1) * NQ],
                    start=True,
                    stop=True,
                )
                nc.vector.tensor_scalar_add(
                    qT_all[:, (b * 4 + p) * NQ : (b * 4 + p + 1) * NQ],
                    ps[:],
                    bq_sb[:, p : p + 1],
                )
                if cap <= 512:
                    pieces = [(0, cap)]
                else:
                    half = (cap // 2 + 63) // 64 * 64
                    pieces = [(0, half), (half, cap)]
                for n0, n1 in pieces:
                    psk = ps_proj.tile([128, NQ], F32, tag="ps")
                    nc.tensor.matmul(
                        psk[:, : n1 - n0],
                        wk_slice(p),
                        keyT_sb[:, KCUM[b] + n0 : KCUM[b] + n1],
                        start=True,
                        stop=True,
                    )
                    nc.vector.tensor_scalar_add(
                        kT_all[
                            :,
                            4 * KCUM[b] + p * cap + n0 : 4 * KCUM[b] + p * cap + n1,
                        ],
                        psk[:, : n1 - n0],
                        bk_sb[:, p : p + 1],
                    )

            def emit_v_proj(b, c0=0, c1=None):
                for c in range(c0, CH[b] if c1 is None else c1):
                    tail = c == CH[b] - 1
                    ps = ps_proj.tile([128, NQ], F32, tag="ps")
                    nc.tensor.matmul(
                        ps[:],
                        keyT_sb[:, KCUM[b] + c * 128 : KCUM[b] + (c + 1) * 128],
                        wv_sb,
                        start=True,
                        stop=True,
                    )
                    bv_ap = p2s(f"bvt{b}") if tail else p2s("bvfull")
                    nc.vector.tensor_add(
                        v_view[:, CCUM[b] + c, :, 0:64],
                        ps[:].rearrange("p (h d) -> p h d", d=64),
                        bv_ap.rearrange("p (h d) -> p h d", d=64),
                    )

            # ---- attention ----
            # Score chunk GROUPS of up to 2 chunks share one PSUM tile and
            # one Exp instruction (bias-free thanks to the compaction).
            # One score "group" = half-chunks [j0, j1) (j = chunk*2 + head
            # parity; the exps layout is linear in j so PV indexing never
            # changes).  Groups are normally whole chunks (j aligned, 2 wide).
            def emit_score_group(exps, b, p, j0, j1):
                st = ps_s.tile([128, 1024], F32)
                qbase = (b * 4 + p) * NQ
                for j in range(j0, j1):
                    c, hh = divmod(j, 2)
                    kbase = 4 * KCUM[b] + p * CAPS[b] + c * 128
                    rows = slice(64 * hh, 64 * hh + 64)
                    nc.tensor.matmul(
                        st[:, (j - j0) * NQ : (j - j0 + 1) * NQ],
                        kT_all[rows, kbase : kbase + 128],
                        qT_all[rows, qbase : qbase + NQ],
                        start=True,
                        stop=True,
                        tile_position=(64 * hh, 0),
                    )
                nc.scalar.activation(
                    exps[:, j0 * NQ : j1 * NQ],
                    st[:, 0 : (j1 - j0) * NQ],
                    mybir.ActivationFunctionType.Exp,
                )

            # Per-pair output staging: both heads' normalized tiles gather in
            # one SBUF buffer, written out with one DMA per pair so the final
            # pair's store is small and the earlier ones overlap compute.
            stage = {}

            ctt = {}

            def emit_pv_mm(exps, b, p, hh):
                h = 2 * p + hh
                ct_ps = ps_pv.tile([65, NQ], F32)
                for c in range(CH[b]):
                    vbase = (CCUM[b] + c) * 520 + h * 65
                    nc.tensor.matmul(
                        ct_ps[:],
                        v_all[:, vbase : vbase + 65],
                        exps[:, c * 1024 + hh * NQ : c * 1024 + hh * NQ + NQ],
                        start=(c == 0),
                        stop=(c == CH[b] - 1),
                    )
                if hh == 0:
                    ct_pair = ctp.tile([65, 2 * NQ], FP16)
                    ctt[(b, p)] = ct_pair
                ct_sb = ctt[(b, p)][:, hh * NQ : (hh + 1) * NQ]
                nc.vector.tensor_copy(ct_sb, ct_ps[:])
                if hh == 1:
                    del ctt[(b, p)]
                return ct_sb

            def emit_pv_tail(ct_sb, b, p, hh):
                tr_ps = ps_proj.tile([128, 4 * 66], FP16, tag="ps")
                trv = tr_ps[:].rearrange("p (q x) -> p q x", x=66)
                for qt in range(4):
                    nc.tensor.transpose(
                        tr_ps[:, qt * 66 : qt * 66 + 65],
                        ct_sb[:, qt * 128 : (qt + 1) * 128],
                        ident_sb[0:65, 0:65],
                    )
                if hh == 0:
                    rec_tile = recp.tile([128, 8], FP16, tag="rec")
                    stage[("r", b)] = rec_tile
                rec = stage[("r", b)][:, hh * 4 : hh * 4 + 4]
                with nc.allow_low_precision(
                    reason="softmax denom reciprocal in fp16; rel err ~1e-3 ok"
                ):
                    nc.vector.reciprocal(rec, trv[:, :, 64])
                if hh == 0:
                    stage_tile = cp.tile([128, 4 * 128], FP16, tag="stage")
                    stage[b] = stage_tile
                cv = stage[b][:].rearrange("p (q d) -> p q d", d=128)
                nc.vector.tensor_mul(
                    cv[:, :, hh * 64 : (hh + 1) * 64],
                    trv[:, :, 0:64],
                    rec.unsqueeze(2).broadcast_to([128, 4, 64]),
                )
                if hh == 1:
                    outq = nc.sync if b == 0 else nc.scalar
                    outq.dma_start(out_d[b, p], stage[b][:])
                    del stage[b]
                    del stage[("r", b)]

            pair_seq = [(b, p) for b in border for p in range(4)]
            NP_ = len(pair_seq)
            # Group partitions (in half-chunk units).  The final pair's last
            # chunk is split per head so the two output tail-chains overlap.
            groups = []
            for b, p in pair_seq:
                J = 2 * CH[b]
                groups.append([(j, min(j + 2, J)) for j in range(0, J, 2)])
            j0, j1 = groups[-1][-1]
            if j1 - j0 == 2:
                groups[-1] = groups[-1][:-1] + [(j0, j0 + 1), (j0 + 1, j1)]

            exps_list = [None] * NP_

            def get_exps(i):
                if exps_list[i] is None:
                    et = expsp.tile(
                        [128, CH[pair_seq[i][0]] * 1024], FP16, tag="exps"
                    )
                    exps_list[i] = et
                return exps_list[i]

            # Interleave the tensor-queue work between score groups so the
            # Exp stream on the scalar engine never starves; each pair's
            # FIRST group is emitted at the tail of the previous pair so the
            # stream never bubbles at pair boundaries.
            emit_qk_proj(*pair_seq[0])
            e0 = get_exps(0)
            for j0, j1 in groups[0]:
                emit_score_group(e0, *pair_seq[0], j0, j1)
            emit_qk_proj(*pair_seq[1])
            emit_score_group(get_exps(1), *pair_seq[1], *groups[1][0])
            for i in range(1, NP_):
                b, p = pair_seq[i]
                exps = get_exps(i)
                gs = groups[i][1:]
                prev = (exps_list[i - 1], *pair_seq[i - 1])

                def sc(k):
                    if k < len(gs):
                        emit_score_group(exps, b, p, *gs[k])

                sc(0)
                if i >= 2 and pair_seq[i - 1][0] != b:
                    # Second half of this batch's V projection, emitted after
                    # the next pair's qk casts so it never delays them.
                    emit_v_proj(b, CH[b] // 2)
                if i == 1:
                    # First batch's V projection: emitted here (not at i==0)
                    # so its DVE adds queue after pair 2's qk casts.  All of
                    # pair 1's score groups go first: the PV matmuls below
                    # wait on these V adds and would stall the PE queue.
                    emit_qk_proj(*pair_seq[2])
                    emit_vcol(b)
                    emit_v_proj(b)
                    for k in range(1, len(gs)):
                        sc(k)
                    ct0 = emit_pv_mm(*prev, 0)
                    emit_pv_tail(ct0, prev[1], prev[2], 0)
                    ct1 = emit_pv_mm(*prev, 1)
                    emit_pv_tail(ct1, prev[1], prev[2], 1)
                else:
                    sc(1)
                    if i + 1 < NP_:
                        emit_qk_proj(*pair_seq[i + 1])
                    ct0 = emit_pv_mm(*prev, 0)
                    sc(2)
                    emit_pv_tail(ct0, prev[1], prev[2], 0)
                    sc(3)
                    ct1 = emit_pv_mm(*prev, 1)
                    emit_pv_tail(ct1, prev[1], prev[2], 1)
                    for k in range(4, len(gs)):
                        sc(k)
                if i + 1 < NP_:
                    emit_score_group(
                        get_exps(i + 1), *pair_seq[i + 1], *groups[i + 1][0]
                    )
                if i + 2 < NP_ and pair_seq[i + 2][0] != b:
                    nb = pair_seq[i + 2][0]
                    emit_vcol(nb)
                    emit_v_proj(nb, 0, CH[nb] // 2)
            prev = (exps_list[NP_ - 1], *pair_seq[NP_ - 1])
            ct0 = emit_pv_mm(*prev, 0)
            emit_pv_tail(ct0, prev[1], prev[2], 0)
            ct1 = emit_pv_mm(*prev, 1)
            emit_pv_tail(ct1, prev[1], prev[2], 1)

    nc.compile()
    return nc


def _prep_host(query, key, c_mask, Wq, bq, Wk, bk, Wv, bv):
    query = np.asarray(query, dtype=np.float32)
    key = np.asarray(key, dtype=np.float32)
    c_mask = np.asarray(c_mask, dtype=np.float32)
    Wq = np.asarray(Wq, dtype=np.float32)
    bq = np.asarray(bq, dtype=np.float32)
    Wk = np.asarray(Wk, dtype=np.float32)
    bk = np.asarray(bk, dtype=np.float32)
    Wv = np.asarray(Wv, dtype=np.float32)
    bv = np.asarray(bv, dtype=np.float32)

    counts = c_mask.sum(axis=1).astype(np.int64)
    # Slot assignment: sort batches by count; smallest N_CORES to slot 0 etc.
    order = np.argsort(counts, kind="stable")
    slot_batches = [order[s * N_CORES : (s + 1) * N_CORES] for s in range(B_LOC)]
    chunk_cfg = tuple(
        max(1, int(math.ceil(int(counts[sb].max()) / 128))) for sb in slot_batches
    )
    CAPS = [c * 128 for c in chunk_cfg]

    queryT = np.ascontiguousarray(query.transpose(0, 2, 1))  # [B, CQ, NQ] f32

    wq_s = (Wq / np.float32(SCALE)).astype(np.float32)
    bq_s = (bq / np.float32(SCALE)).astype(np.float32)

    wq16 = wq_s.astype(NP_FP16)
    wk16 = Wk.astype(NP_FP16)
    pack1 = np.ascontiguousarray(
        np.concatenate(
            [
                wq16[:, 0:128],
                wk16[:, 0:128],
                bq_s.reshape(4, 128).T.astype(NP_FP16),
                bk.reshape(4, 128).T.astype(NP_FP16),
            ],
            axis=1,
        )
    )
    ident = np.eye(128, dtype=NP_FP16)
    bv_full = np.broadcast_to(bv, (128, HD)).astype(NP_FP16)
    in_maps = []
    assignment = []  # (core, slot) -> batch index
    for core in range(N_CORES):
        m = {"pack1": pack1}
        keyT_parts = []
        cmask_parts = []
        qT_parts = []
        bv_blocks = [bv_full]
        batches = []
        for s in range(B_LOC):
            b = int(slot_batches[s][core])
            batches.append(b)
            cap = CAPS[s]
            cnt = int(counts[b])
            perm = np.argsort(1.0 - c_mask[b], kind="stable")[:cap]
            kt = key[b][perm].T.astype(NP_FP16)  # [CV, cap]
            kt[:, cnt:] = 0  # zero the pad keys
            keyT_parts.append(kt)
            cm = np.zeros((cap,), dtype=NP_FP16)
            cm[:cnt] = 1.0
            cmask_parts.append(cm.reshape(chunk_cfg[s], 128).T)  # [128, ch]
            # tail-chunk bv with pad partitions zeroed
            tail_valid = cnt - (chunk_cfg[s] - 1) * 128
            bvt = bv_full.copy()
            bvt[max(tail_valid, 0) :, :] = 0
            bv_blocks.append(bvt)
            qT_parts.append(queryT[b].astype(NP_FP16))
        m["queryT"] = np.ascontiguousarray(np.concatenate(qT_parts, axis=1))
        m["keyT"] = np.ascontiguousarray(np.concatenate(keyT_parts, axis=1))
        parts = {
            "colmask": np.concatenate(cmask_parts, axis=1),
            "wv": Wv.astype(NP_FP16),
            "bvfull": bv_blocks[0],
            "ident": ident,
        }
        for p_ in range(1, 4):
            parts[f"wq{p_}"] = wq16[:, p_ * 128 : (p_ + 1) * 128]
            parts[f"wk{p_}"] = wk16[:, p_ * 128 : (p_ + 1) * 128]
        for s in range(B_LOC):
            parts[f"bvt{s}"] = bv_blocks[1 + s]
        m["pack2"] = np.ascontiguousarray(
            np.concatenate(
                [parts[nm] for nm, _ in PACK2_ORDER(len(parts["colmask"][0]))],
                axis=1,
            )
        )
        in_maps.append(m)
        assignment.append(batches)
    return chunk_cfg, in_maps, assignment


def kernel(query, key, c_mask, Wq, bq, Wk, bk, Wv, bv):
    global LAST_EXEC_TIME_NS
    chunk_cfg, in_maps, assignment = _prep_host(
        query, key, c_mask, Wq, bq, Wk, bk, Wv, bv
    )
    if chunk_cfg not in _PROGRAM_CACHE:
        _PROGRAM_CACHE[chunk_cfg] = _build_program(chunk_cfg)
    nc = _PROGRAM_CACHE[chunk_cfg]
    res = run_bass_kernel_spmd(
        nc,
        in_maps,
        core_ids=list(range(N_CORES)),
        trace=bool(os.environ.get("BASS_TRACE")),
    )
    LAST_EXEC_TIME_NS = res.exec_time_ns
    out = np.empty((B, NQ, HD), dtype=np.float32)
    for core in range(N_CORES):
        # [b, p, i, t, d] -> [b, (t i), (p d)]
        o = (
            np.asarray(res.results[core]["out"], dtype=np.float32)
            .reshape(B_LOC, 4, 128, 4, 128)
            .transpose(0, 3, 2, 1, 4)
            .reshape(B_LOC, NQ, HD)
        )
        for s in range(B_LOC):
            out[assignment[core][s]] = o[s]
    return out
